# revision 1
# baseline (speedup 1.0000x reference)
"""Trainium2 Bass kernel for the non-local-attention block (nn_DNL_74234214744693).

Reference computation (B=4, C=64, H=W=64, N=H*W=4096):
    k = conv1x1(x,kw,kb); k_wh = k - mean_j(k)
    q = conv1x1(x,qw,qb); q_wh = q - mean_j(q)
    qk[b,i,j] = sum_c k_wh[b,c,i] q_wh[b,c,j]
    m  = conv1x1(x,mw,mb) -> [B,N];  mm[b,i,j] = m[b,i]*m[b,j]
    f  = softmax(qk, axis=-1) + softmax(mm, axis=0)   # second softmax over BATCH
    y  = einsum('bci,bij->bcj', v, f) + BN(conv1x1(x,ww,wb))

Key algebraic facts used:
  * softmax_j(k_whT q_wh) == softmax_j(k_whT q_raw): the q-mean term is constant
    along j's softmax rows, so only k needs whitening.
  * softmax_j normalizer Z1[i] indexes the contraction dim, so y1 = (v/Z1) @ e1.
  * batch softmax: f2[b] = e2_b * R with e2_b = exp(m_b_i m_b_j),
    R = 1/D = exp(-ln(sum_b e2_b)) (exp+ln live in one ACT table set).

Sharding: each of 8 cores owns a 512-row i-slice of the [N,N] maps for ALL 4
batch samples (exp work is perfectly balanced, no duplication, no collectives).
Each core emits a partial y [4,64,4096]; host sums the 8 partials.
The conv+BN residual is folded into the output matmul with weights pre-scaled
by 1/8 (so the host-side sum reconstructs it exactly once).
"""

import functools

import numpy as np
import ml_dtypes

N_CORES = 8
B, C, H, W = 4, 64, 64, 64
N = H * W                 # 4096
SL = N // N_CORES         # 512  rows of the attention map per core
NIT = SL // 128           # 4    128-row tiles per core
NJQ = 4                   # 1024-wide column blocks in phase B
JQ = N // NJQ             # 1024
EPS = 1e-5

BF16 = ml_dtypes.bfloat16


def _build_program():
    import concourse.bass as bass
    import concourse.tile as tile
    from concourse import bacc, mybir

    dt = mybir.dt
    AF = mybir.ActivationFunctionType
    ALU = mybir.AluOpType
    AX = mybir.AxisListType

    nc = bacc.Bacc("TRN2", target_bir_lowering=False, debug=False,
                   enable_asserts=False, num_devices=1)

    # ---------------- DRAM I/O ----------------
    x_ext = nc.dram_tensor("x_ext", [B, C + 1, N], dt.bfloat16, kind="ExternalInput")
    xsl_ext = nc.dram_tensor("xsl_ext", [B, C + 1, SL], dt.bfloat16, kind="ExternalInput")
    qmT = nc.dram_tensor("qmT", [C + 1, C + 1], dt.bfloat16, kind="ExternalInput")
    kT = nc.dram_tensor("kT", [C + 1, C], dt.bfloat16, kind="ExternalInput")
    vmT = nc.dram_tensor("vmT", [C + 1, C + 1], dt.bfloat16, kind="ExternalInput")
    wT = nc.dram_tensor("wT", [C + 1, C], dt.bfloat16, kind="ExternalInput")
    y_part = nc.dram_tensor("y_part", [B, C, N], dt.float32, kind="ExternalOutput")

    with tile.TileContext(nc) as tc:
        from contextlib import ExitStack

        with ExitStack() as top:
            # ---------- persistent pools ----------
            consts = top.enter_context(tc.tile_pool(name="consts", bufs=1))
            p_kwh = top.enter_context(tc.tile_pool(name="p_kwh", bufs=B))
            p_vT = top.enter_context(tc.tile_pool(name="p_vT", bufs=B * NIT))
            p_v1p = top.enter_context(tc.tile_pool(name="p_v1p", bufs=B * NIT))
            p_mcol = top.enter_context(tc.tile_pool(name="p_mcol", bufs=B * NIT))
            p_f1 = top.enter_context(tc.tile_pool(name="p_f1", bufs=B * NIT))
            p_small = top.enter_context(tc.tile_pool(name="p_small", bufs=B * 4))
            dram = top.enter_context(tc.tile_pool(name="dram", bufs=1, space="DRAM"))

            sb_qmT = consts.tile([C + 1, C + 1], dt.bfloat16)
            sb_kT = consts.tile([C + 1, C], dt.bfloat16)
            sb_vmT = consts.tile([C + 1, C + 1], dt.bfloat16)
            sb_wT = consts.tile([C + 1, C], dt.bfloat16)
            nc.sync.dma_start(sb_qmT, qmT.ap())
            nc.sync.dma_start(sb_kT, kT.ap())
            nc.sync.dma_start(sb_vmT, vmT.ap())
            nc.sync.dma_start(sb_wT, wT.ap())

            md = dram.tile([B, N], dt.bfloat16)  # m values, for broadcast DMA

            k_wh = [p_kwh.tile([C, SL], dt.bfloat16, name=f"k_wh{b}", tag="k_wh") for b in range(B)]
            v_T = [[p_vT.tile([128, C], dt.bfloat16, name=f"v_T{b}_{i}", tag="v_T") for i in range(NIT)] for b in range(B)]
            v1p = [[p_v1p.tile([128, C], dt.bfloat16, name=f"v1p{b}_{i}", tag="v1p") for i in range(NIT)] for b in range(B)]
            m_col = [[p_mcol.tile([128, 1], dt.float32, name=f"m_col{b}_{i}", tag="m_col") for i in range(NIT)] for b in range(B)]
            f1 = [[p_f1.tile([128, N], dt.bfloat16, name=f"f1_{b}_{i}", tag="f1") for i in range(NIT)] for b in range(B)]
            negku = [p_small.tile([C, 1], dt.float32, name=f"negku{b}", tag="negku") for b in range(B)]

            # ---------- phases 0+A interleaved per b: convs then qk/exp ----------
            with ExitStack() as ph0:
                p_q = ph0.enter_context(tc.tile_pool(name="p_q", bufs=2))
                p_x = ph0.enter_context(tc.tile_pool(name="p_x", bufs=2))
                p_xsl = ph0.enter_context(tc.tile_pool(name="p_xsl", bufs=2))
                psP = ph0.enter_context(tc.tile_pool(name="psP", bufs=2, space="PSUM"))
                p_t0 = ph0.enter_context(tc.tile_pool(name="p_t0", bufs=8))
                p_z = ph0.enter_context(tc.tile_pool(name="p_z", bufs=8))

                def dma_phase(b):
                    x_sb = p_x.tile([C + 1, N], dt.bfloat16, name=f"x_sb{b}", tag="x_sb")
                    xsl_sb = p_xsl.tile([C + 1, SL], dt.bfloat16, name=f"xsl_sb{b}", tag="xsl_sb")
                    nc.sync.dma_start(x_sb, x_ext.ap()[b])
                    nc.sync.dma_start(xsl_sb, xsl_ext.ap()[b])
                    return x_sb, xsl_sb

                def q_block(b, x_sb, q_store):
                    for half in range(2):
                        ps_q = psP.tile([128, 2048], dt.float32, name=f"ps_q{b}_{half}", tag="psP")
                        for k4 in range(4):
                            j0 = half * 2048 + k4 * 512
                            nc.tensor.matmul(ps_q[0:C + 1, k4 * 512:(k4 + 1) * 512],
                                             sb_qmT, x_sb[:, j0:j0 + 512],
                                             start=True, stop=True)
                        dst = q_store[:, half * 2048:(half + 1) * 2048]
                        nc.vector.tensor_copy(dst, ps_q[0:C + 1, :])

                def conv_phase(b, x_sb, xsl_sb):
                    q_store = p_q.tile([C + 1, N], dt.bfloat16, name=f"q_store{b}", tag="q_store")

                    # xu = mean_j(x) (row 64 = ones -> mean 1.0)
                    xu_f = p_t0.tile([C + 1, 1], dt.float32, tag="t0")
                    xu_bf = p_t0.tile([C + 1, 1], dt.bfloat16, tag="t0b")
                    nc.vector.tensor_reduce(xu_f, x_sb, axis=AX.X, op=ALU.add)
                    nc.vector.tensor_scalar_mul(xu_bf, xu_f, 1.0 / N)

                    # misc psum slot: ku + v/m convs + k conv packed into one tile
                    ps_m = psP.tile([128, 2048], dt.float32, name=f"ps_m{b}", tag="psP")
                    nc.tensor.matmul(ps_m[0:C, 1536:1537], sb_kT, xu_bf,
                                     start=True, stop=True)
                    nc.vector.tensor_scalar_mul(negku[b], ps_m[0:C, 1536:1537], -1.0)
                    for it in range(NIT):
                        fo = (it // 2) * 512 + (it % 2) * 256
                        nc.tensor.matmul(ps_m[:, fo:fo + C + 1],
                                         xsl_sb[:, it * 128:(it + 1) * 128],
                                         sb_vmT, start=True, stop=True)
                    nc.tensor.matmul(ps_m[0:C, 1024:1536], sb_kT, xsl_sb,
                                     start=True, stop=True)
                    for it in range(NIT):
                        fo = (it // 2) * 512 + (it % 2) * 256
                        nc.vector.tensor_copy(v_T[b][it], ps_m[:, fo:fo + C])
                        nc.vector.tensor_copy(m_col[b][it], ps_m[:, fo + C:fo + C + 1])
                    nc.vector.tensor_scalar(k_wh[b], ps_m[0:C, 1024:1536],
                                            scalar1=negku[b], scalar2=None, op0=ALU.add)

                    # q_raw (rows 0..63) and m_row (row 64)
                    q_block(b, x_sb, q_store)

                    # stash m (bf16) in DRAM for later broadcast DMA
                    nc.sync.dma_start(md[b], q_store[C:C + 1, :])
                    return q_store

                def qk_phase(b, q_store, its):
                    # qk -> e1 (bf16) + row sums -> v1p
                    for it in its:
                        zp = [p_z.tile([128, 1], dt.float32, name=f"zp{j}", tag="zp") for j in range(2)]
                        for jh in range(2):
                            ps_qk = psP.tile([128, 2048], dt.float32, name="ps_qk", tag="psP")
                            for k4 in range(4):
                                j0 = jh * 2048 + k4 * 512
                                nc.tensor.matmul(
                                    ps_qk[:, k4 * 512:(k4 + 1) * 512],
                                    k_wh[b][:, it * 128:(it + 1) * 128],
                                    q_store[0:C, j0:j0 + 512],
                                    start=True, stop=True)
                            nc.scalar.activation(
                                f1[b][it][:, jh * 2048:(jh + 1) * 2048],
                                ps_qk, AF.Exp, accum_out=zp[jh])
                        z1 = p_z.tile([128, 1], dt.float32)
                        rz = p_z.tile([128, 1], dt.float32)
                        nc.vector.tensor_tensor(z1, zp[0], zp[1], op=ALU.add)
                        nc.vector.reciprocal_approx_fast(rz, z1)
                        nc.vector.tensor_scalar_mul(v1p[b][it], v_T[b][it], rz)

                # per-b: convs then qk; conv(b+1) is emitted between
                # qk(b)'s it=0..2 and it=3 so its psum slots and DVE copies
                # complete under the last e1 exps (kills the ~4.4us ACT gap
                # at each b transition); its DMAs are issued a phase early.
                dmas_cur = dma_phase(0)
                q_cur = conv_phase(0, *dmas_cur)
                for b in range(B):
                    if b + 1 < B:
                        dmas_next = dma_phase(b + 1)
                        qk_phase(b, q_cur, range(NIT - 1))
                        q_next = conv_phase(b + 1, *dmas_next)
                        qk_phase(b, q_cur, [NIT - 1])
                        q_cur = q_next
                    else:
                        qk_phase(b, q_cur, range(NIT))

            # ---------- phase B: e2/D/R/f2 + output matmuls ----------
            with ExitStack() as phB:
                psY = phB.enter_context(tc.tile_pool(name="psY", bufs=8, space="PSUM"))
                p_mbc = phB.enter_context(tc.tile_pool(name="p_mbc", bufs=6))
                p_e2 = phB.enter_context(tc.tile_pool(name="p_e2", bufs=14))
                p_dr = phB.enter_context(tc.tile_pool(name="p_dr", bufs=2))
                p_rr = phB.enter_context(tc.tile_pool(name="p_rr", bufs=1))
                p_rb = phB.enter_context(tc.tile_pool(name="p_rb", bufs=2))
                p_xw = phB.enter_context(tc.tile_pool(name="p_xw", bufs=5))
                p_out = phB.enter_context(tc.tile_pool(name="p_out", bufs=2))

                for jq in range(NJQ):
                    jsl = slice(jq * JQ, (jq + 1) * JQ)
                    m_bc = []
                    for b in range(B):
                        t = p_mbc.tile([128, JQ], dt.bfloat16, name="m_bc", tag="m_bc")
                        nc.sync.dma_start(t, md[b:b + 1, jsl].to_broadcast([128, JQ]))
                        m_bc.append(t)
                    x_wx = []
                    for b in range(B):
                        t = p_xw.tile([C + 1, JQ], dt.bfloat16, name="x_wx", tag="x_wx")
                        nc.sync.dma_start(t, x_ext.ap()[b][:, jsl])
                        x_wx.append(t)

                    ps_y = [[psY.tile([C, 512], dt.float32, name=f"ps_y{b}_{h}", tag="ps_y")
                             for h in range(2)] for b in range(B)]
                    # wx residual first: it is f2-independent, opens each
                    # accumulation group early so the group closes right
                    # after the last f2 matmul (shorter per-jq tail).
                    for b in range(B):
                        for h in range(2):
                            cs = slice(h * 512, (h + 1) * 512)
                            nc.tensor.matmul(ps_y[b][h], sb_wT, x_wx[b][:, cs],
                                             start=True, stop=False)
                    for it in range(NIT):
                        # e2_b = exp(m_i * m_j); D = sum_b e2; R = exp(-ln D);
                        # f2_b = e2_b * R (in place), consumed immediately below.
                        e2 = [p_e2.tile([128, JQ], dt.bfloat16, name=f"e2_{b}", tag="e2") for b in range(B)]
                        for b in range(B):
                            nc.scalar.activation(e2[b], m_bc[b], AF.Exp,
                                                 scale=m_col[b][it])
                        dsum = p_dr.tile([128, JQ], dt.bfloat16)
                        rr = p_rr.tile([128, JQ], dt.float32)
                        nc.vector.tensor_tensor(dsum, e2[0], e2[1], op=ALU.add)
                        nc.vector.tensor_tensor(dsum, dsum, e2[2], op=ALU.add)
                        nc.vector.tensor_tensor(rr, dsum, e2[3], op=ALU.add)
                        nc.vector.reciprocal_approx_fast(rr, rr)
                        rrb = p_rb.tile([128, JQ], dt.bfloat16)
                        nc.vector.tensor_copy(rrb, rr)
                        for b in range(B):
                            eng = nc.vector if b < 2 else nc.gpsimd
                            eng.tensor_tensor(e2[b], e2[b], rrb, op=ALU.mult)
                        for b in range(B):
                            for h in range(2):
                                cs = slice(h * 512, (h + 1) * 512)
                                js = slice(jq * JQ + h * 512, jq * JQ + (h + 1) * 512)
                                nc.tensor.matmul(ps_y[b][h], v1p[b][it],
                                                 f1[b][it][:, js],
                                                 start=False, stop=False)
                                nc.tensor.matmul(ps_y[b][h], v_T[b][it],
                                                 e2[b][:, cs],
                                                 start=False,
                                                 stop=(it == NIT - 1))

                    for b in range(B):
                        out_sb = p_out.tile([C, JQ], dt.float32)
                        for h in range(2):
                            cs = slice(h * 512, (h + 1) * 512)
                            nc.scalar.copy(out_sb[:, cs], ps_y[b][h])
                        nc.sync.dma_start(y_part.ap()[b][:, jsl], out_sb)

    nc.compile()
    return nc


@functools.lru_cache(maxsize=1)
def _get_program():
    return _build_program()


def _prep_inputs(inputs):
    x = np.asarray(inputs["x"], np.float32).reshape(B, C, N)
    ones = np.ones((B, 1, N), np.float32)
    x_ext = np.concatenate([x, ones], axis=1).astype(BF16)          # [B,65,N]

    qw = np.asarray(inputs["qw"], np.float32)
    qb = np.asarray(inputs["qb"], np.float32)
    kw = np.asarray(inputs["kw"], np.float32)
    kb = np.asarray(inputs["kb"], np.float32)
    mw = np.asarray(inputs["mw"], np.float32)
    mb = np.asarray(inputs["mb"], np.float32)
    vw = np.asarray(inputs["vw"], np.float32)
    vb = np.asarray(inputs["vb"], np.float32)
    ww = np.asarray(inputs["ww"], np.float32)
    wb = np.asarray(inputs["wb"], np.float32)
    g = np.asarray(inputs["bn_gamma"], np.float32)
    be = np.asarray(inputs["bn_beta"], np.float32)
    rm = np.asarray(inputs["bn_rm"], np.float32)
    rv = np.asarray(inputs["bn_rv"], np.float32)

    qmT = np.zeros((C + 1, C + 1), np.float32)
    qmT[:C, :C] = qw.T
    qmT[C, :C] = qb
    qmT[:C, C] = mw[0]
    qmT[C, C] = mb[0]

    kT = np.concatenate([kw.T, kb[None, :]], axis=0)                # [65,64]

    vmT = np.zeros((C + 1, C + 1), np.float32)
    vmT[:C, :C] = vw.T
    vmT[C, :C] = vb
    vmT[:C, C] = mw[0]
    vmT[C, C] = mb[0]

    inv = g / np.sqrt(rv + EPS)
    wT = np.zeros((C + 1, C), np.float32)
    wT[:C, :] = (ww * inv[:, None]).T / N_CORES
    wT[C, :] = (wb * inv + be - rm * inv) / N_CORES

    common = {
        "x_ext": x_ext,
        "qmT": qmT.astype(BF16),
        "kT": kT.astype(BF16),
        "vmT": vmT.astype(BF16),
        "wT": wT.astype(BF16),
    }
    in_maps = []
    for ic in range(N_CORES):
        m = dict(common)
        m["xsl_ext"] = np.ascontiguousarray(x_ext[:, :, ic * SL:(ic + 1) * SL])
        in_maps.append(m)
    return in_maps


def kernel(**inputs):
    from concourse.bass_utils import run_bass_kernel_spmd

    nc = _get_program()
    in_maps = _prep_inputs(inputs)
    res = run_bass_kernel_spmd(nc, in_maps, core_ids=list(range(N_CORES)))
    y = np.zeros((B, C, N), np.float32)
    for r in res.results:
        y += r["y_part"]
    return y.reshape(B, C, H, W)


if __name__ == "__main__":
    rng = np.random.default_rng(0)
    ins = {
        "x": rng.standard_normal((B, C, H, W), dtype=np.float32),
        "qw": rng.standard_normal((C, C), dtype=np.float32) * 0.05,
        "qb": rng.standard_normal((C,), dtype=np.float32) * 0.05,
        "kw": rng.standard_normal((C, C), dtype=np.float32) * 0.05,
        "kb": rng.standard_normal((C,), dtype=np.float32) * 0.05,
        "mw": rng.standard_normal((1, C), dtype=np.float32) * 0.05,
        "mb": rng.standard_normal((1,), dtype=np.float32) * 0.05,
        "vw": rng.standard_normal((C, C), dtype=np.float32) * 0.05,
        "vb": rng.standard_normal((C,), dtype=np.float32) * 0.05,
        "ww": rng.standard_normal((C, C), dtype=np.float32) * 0.05,
        "wb": rng.standard_normal((C,), dtype=np.float32) * 0.05,
        "bn_gamma": np.ones((C,), np.float32),
        "bn_beta": np.zeros((C,), np.float32),
        "bn_rm": np.zeros((C,), np.float32),
        "bn_rv": np.ones((C,), np.float32),
    }
    out = kernel(**ins)
    print("kernel output", out.shape, out.dtype, np.abs(out).mean())

